# revision 1
# baseline (speedup 1.0000x reference)
"""Trainium2 Bass kernel for nn_MultiHeadAttention_63015760167496.

Computation (see reference): qkv = x @ Wqkv; RoPE on q,k; causal softmax
attention per head; out = einsum('bhts,bshd->bhtd', probs, v);
out.reshape(B,T,C) @ Wout  -- NOTE the reshape is a *head-major* flatten of
[B,H,T,D] into [B,T,C], so final-output row r = h*128 + t//16 depends only on
head h.  Sharding: head-parallel over 8 cores (2 heads/core); every core
computes its two heads end-to-end and produces final-output rows
[256*i, 256*i+256).  Host concatenates -- no collectives.

All big matmuls run as float32r (TF32-like) on the tensor engine.
Attention is computed in S^T layout ([s,t]): softmax denominator via a
ones-vector matmul (partition reduction on the PE), normalization via a K=1
broadcast matmul.  No running max is needed (scores are O(5), fp32 psum).
Host pre-arranges x^T and the weight slices so every DMA is 128 partitions
x >=16KB contiguous.
"""

import math
import sys

for _p in ("/opt/trn_rl_repo", "/root/.axon_site/_ro/trn_rl_repo"):
    if _p not in sys.path:
        sys.path.insert(0, _p)

import numpy as np

import concourse.bass as bass
import concourse.mybir as mybir
import concourse.tile as tile
from concourse import bacc
from concourse.bass_utils import run_bass_kernel_spmd

B, T, C = 2, 2048, 2048
H = 16            # heads total
D = C // H        # 128 head dim
HALF = D // 2     # 64
P = 128
KO = C // P       # 16 contraction chunks
NCORES = 8
HPC = H // NCORES  # 2 heads per core
TQ = 256          # t-tile for qkv projection
NT = T // TQ
TA = 256          # t-tile for attention
NSC = T // P      # 16 s-chunks
ROPE_BASE = 10000.0
SCALE = 1.0 / math.sqrt(D)

f32 = mybir.dt.float32
f32r = mybir.dt.float32r


def _build():
    nc = bacc.Bacc("TRN2", target_bir_lowering=False, debug=False,
                   num_devices=NCORES)

    # host-pre-tiled x^T: xTt[b, ti, p, ko, u] = x[b, ti*TQ+u, ko*128+p]
    xTt = nc.dram_tensor("xTt", [B, NT, P, KO, TQ], f32r, kind="ExternalInput")
    # host-pre-chunked weights: w[p, ko, m] = W[ko*128+p, m]
    wq = nc.dram_tensor("wq", [P, KO, HPC * D], f32r, kind="ExternalInput")
    wk = nc.dram_tensor("wk", [P, KO, HPC * D], f32r, kind="ExternalInput")
    wv = nc.dram_tensor("wv", [P, KO, HPC * D], f32r, kind="ExternalInput")
    wout = nc.dram_tensor("wout", [C, C], f32r, kind="ExternalInput")
    cs2 = nc.dram_tensor("cs2", [P, T], f32, kind="ExternalInput")  # [cos;cos]
    sn1 = nc.dram_tensor("sn1", [HALF, T], f32, kind="ExternalInput")  # sin
    maskM = nc.dram_tensor("maskM", [P, P], f32r, kind="ExternalInput")
    y = nc.dram_tensor("y", [B, HPC * D, C], f32, kind="ExternalOutput")

    with tile.TileContext(nc) as tc:
        with tc.tile_pool(name="const", bufs=1) as cp_, \
             tc.tile_pool(name="qkv", bufs=1) as qp, \
             tc.tile_pool(name="ot", bufs=1) as op_, \
             tc.tile_pool(name="small", bufs=2) as sp:

            wq_sb = cp_.tile([P, KO, HPC * D], f32r, tag="wq")
            wk_sb = cp_.tile([P, KO, HPC * D], f32r, tag="wk")
            wv_sb = cp_.tile([P, KO, HPC * D], f32r, tag="wv")
            nc.sync.dma_start(wq_sb[:], wq.ap())
            cs_sb = cp_.tile([P, T], f32, tag="cs")
            sn_sb = cp_.tile([HALF, T], f32, tag="sn")
            mask_sb = cp_.tile([P, P], f32r, tag="mask")
            ones_row = cp_.tile([1, P], f32, tag="ones_row")
            nc.vector.memset(ones_row[:], 1.0)
            ones_rowr = cp_.tile([1, P], f32r, tag="ones_rowr")
            nc.vector.tensor_copy(ones_rowr[:], ones_row[:])
            ones_f32 = cp_.tile([P, 1], f32, tag="ones_f32")
            nc.vector.memset(ones_f32[:], 1.0)
            ones_col = cp_.tile([P, 1], f32r, tag="ones_col")
            nc.vector.tensor_copy(ones_col[:], ones_f32[:])

            # persistent attention outputs O^T per (b, local head): [d, t]
            oT = [[op_.tile([P, T], f32r, tag=f"oT{b}{hh}", name=f"oT{b}{hh}")
                   for hh in range(HPC)] for b in range(B)]

            for b in range(B):
                qT = [qp.tile([P, T], f32r, tag=f"qT{hh}", name=f"qT{b}{hh}")
                      for hh in range(HPC)]
                kT = [qp.tile([P, T], f32r, tag=f"kT{hh}", name=f"kT{b}{hh}")
                      for hh in range(HPC)]
                vt = [qp.tile([P, NSC, D], f32r, tag=f"v{hh}", name=f"v{b}{hh}")
                      for hh in range(HPC)]

                # ---------------- QKV projection + RoPE ----------------
                with tc.tile_pool(name=f"xt{b}", bufs=2) as xp, \
                     tc.tile_pool(name=f"psA{b}", bufs=4, space="PSUM") as psa, \
                     tc.tile_pool(name=f"rope{b}", bufs=3) as rp:
                    for ti in range(NT):
                        sl = slice(ti * TQ, (ti + 1) * TQ)
                        xt = xp.tile([P, KO, TQ], f32r, tag="xt")
                        nc.sync.dma_start(xt[:], xTt.ap()[b, ti])
                        cs = cs_sb[:, sl]
                        sn = sn_sb[:, sl]  # [64, TQ] base partition 0

                        def qkmm(w_sb, hh):
                            hsl = slice(hh * D, (hh + 1) * D)
                            ps = psa.tile([P, TQ], f32, tag="acc",
                                          name=f"acc{b}_{ti}_{hh}")
                            for ko in range(KO):
                                nc.tensor.matmul(ps[:], w_sb[:, ko, hsl],
                                                 xt[:, ko, :],
                                                 start=(ko == 0),
                                                 stop=(ko == KO - 1))
                            return ps

                        def rope(ps, dst):
                            # tcos = ps * [cos;cos] (one full mult); tsw
                            # pre-swaps halves: tsw[0:64]=q2*sin,
                            # tsw[64:128]=q1*sin so the gpsimd add/sub reads
                            # align on base partitions.
                            tcos = rp.tile([P, TQ], f32, tag="tcos")
                            tsw = rp.tile([P, TQ], f32, tag="tsw")
                            nc.vector.tensor_mul(tcos[:], ps[:], cs)
                            nc.vector.tensor_mul(tsw[0:HALF, :],
                                                 ps[HALF:P, :], sn)
                            nc.vector.tensor_mul(tsw[HALF:P, :],
                                                 ps[0:HALF, :], sn)
                            nc.gpsimd.tensor_sub(dst[0:HALF, sl],
                                                 tcos[0:HALF, :],
                                                 tsw[0:HALF, :])
                            nc.gpsimd.tensor_add(dst[HALF:P, sl],
                                                 tcos[HALF:P, :],
                                                 tsw[HALF:P, :])

                        if b == 0 and ti == 0:
                            # q accums first (need only wq + xt0); stagger the
                            # remaining const DMAs behind them so the first
                            # matmuls aren't starved for DMA bandwidth.
                            psq = [qkmm(wq_sb, hh) for hh in range(HPC)]
                            nc.sync.dma_start(cs_sb[:], cs2.ap())
                            nc.sync.dma_start(sn_sb[:], sn1.ap())
                            nc.sync.dma_start(wk_sb[:], wk.ap())
                            nc.sync.dma_start(wv_sb[:], wv.ap())
                            nc.sync.dma_start(mask_sb[:], maskM.ap())
                            for hh in range(HPC):
                                rope(psq[hh], qT[hh])
                                psk = qkmm(wk_sb, hh)
                                rope(psk, kT[hh])
                        else:
                            for hh in range(HPC):
                                for w_sb, dst in ((wq_sb, qT[hh]),
                                                  (wk_sb, kT[hh])):
                                    rope(qkmm(w_sb, hh), dst)
                        for sub in range(TQ // P):
                            psv = psa.tile([P, HPC * D], f32, tag="acc")
                            for ko in range(KO):
                                nc.tensor.matmul(
                                    psv[:], xt[:, ko, sub * P:(sub + 1) * P],
                                    wv_sb[:, ko, :],
                                    start=(ko == 0), stop=(ko == KO - 1))
                            tci = ti * (TQ // P) + sub
                            for hh in range(HPC):
                                nc.vector.tensor_copy(
                                    vt[hh][:, tci, :],
                                    psv[:, hh * D:(hh + 1) * D])

                # ---------------- attention (S^T layout) ----------------
                with tc.tile_pool(name=f"psBsc{b}", bufs=3, space="PSUM") as pssc, \
                     tc.tile_pool(name=f"psBo{b}", bufs=2, space="PSUM") as pso, \
                     tc.tile_pool(name=f"psBsum{b}", bufs=2, space="PSUM") as pssum, \
                     tc.tile_pool(name=f"psBbc{b}", bufs=1, space="PSUM") as psbc, \
                     tc.tile_pool(name=f"pt{b}", bufs=3) as ptp:
                    for hh in range(HPC):
                        for ta in range(T // TA):
                            tsl = slice(ta * TA, (ta + 1) * TA)
                            ps_o = pso.tile([P, TA], f32, tag="o")
                            ps_sum = pssum.tile([1, TA], f32, tag="sum")
                            smax = (ta + 1) * (TA // P) - 1
                            for s in range(smax + 1):
                                diag = s >= ta * (TA // P)
                                t_lo = (s - ta * (TA // P)) * P if diag else 0
                                w = slice(t_lo, TA)
                                qsl = slice(ta * TA + t_lo, (ta + 1) * TA)
                                ps_sc = pssc.tile([P, TA], f32, tag="sc")
                                nc.tensor.matmul(
                                    ps_sc[:, w], kT[hh][:, s * P:(s + 1) * P],
                                    qT[hh][:, qsl], start=True, stop=True)
                                pt = ptp.tile([P, TA], f32r, tag="pt")
                                nc.scalar.activation(
                                    pt[:, w], ps_sc[:, w],
                                    mybir.ActivationFunctionType.Exp,
                                    scale=SCALE)
                                if diag:  # mask the 128x128 triangle
                                    nc.vector.tensor_mul(
                                        pt[:, t_lo:t_lo + P],
                                        pt[:, t_lo:t_lo + P], mask_sb[:])
                                first, last = (s == 0), (s == smax)
                                nc.tensor.matmul(ps_o[:, w], vt[hh][:, s, :],
                                                 pt[:, w],
                                                 start=first, stop=last)
                                nc.tensor.matmul(ps_sum[:, w], ones_col[:],
                                                 pt[:, w],
                                                 start=first, stop=last)
                            recf = sp.tile([1, TA], f32, tag="recf")
                            nc.vector.reciprocal_approx_fast(recf[:], ps_sum[:])
                            rec = sp.tile([1, TA], f32r, tag="rec")
                            nc.vector.tensor_copy(rec[:], recf[:])
                            ps_bc = psbc.tile([P, TA], f32, tag="bc")
                            nc.tensor.matmul(ps_bc[:], ones_rowr[:], rec[:],
                                             start=True, stop=True)
                            bc_sb = sp.tile([P, TA], f32, tag="bc_sb")
                            nc.scalar.copy(bc_sb[:], ps_bc[:])
                            # write oT pre-shuffled for the out-projection:
                            # oT[p, j*128+u] = O^T[p, t=u*16+j]
                            oview = oT[b][hh].rearrange(
                                "p (j u) -> p u j", j=KO)[
                                :, (TA // 16) * ta:(TA // 16) * (ta + 1), :]
                            nc.vector.tensor_mul(
                                oview,
                                ps_o[:].rearrange("p (u j) -> p u j", j=KO),
                                bc_sb[:].rearrange("p (u j) -> p u j", j=KO))

            # ---------------- output projection ----------------
            TC_ = 256
            with tc.tile_pool(name="woutp", bufs=1) as wop, \
                 tc.tile_pool(name="psC", bufs=4, space="PSUM") as psc:
                for cpi in range(C // TC_):
                    csl = slice(cpi * TC_, (cpi + 1) * TC_)
                    wts = []
                    for j in range(KO):
                        wt = wop.tile([P, TC_], f32r, tag=f"w{j}",
                                      name=f"w{cpi}_{j}")
                        nc.sync.dma_start(wt[:], wout.ap()[j * P:(j + 1) * P, csl])
                        wts.append(wt)
                    for b in range(B):
                        for hh in range(HPC):
                            psy = psc.tile([P, TC_], f32, tag="y")
                            for j in range(KO):
                                nc.tensor.matmul(psy[:],
                                                 oT[b][hh][:, j * P:(j + 1) * P],
                                                 wts[j][:],
                                                 start=(j == 0),
                                                 stop=(j == KO - 1))
                            ysb = sp.tile([P, TC_], f32, tag="ysb")
                            nc.vector.tensor_copy(ysb[:], psy[:])
                            nc.sync.dma_start(
                                y.ap()[b, hh * D:(hh + 1) * D, csl], ysb[:])

    nc.compile()
    return nc


_NC = None


def _get_nc():
    global _NC
    if _NC is None:
        _NC = _build()
    return _NC


def _host_tables():
    pos = np.arange(T, dtype=np.float32)[:, None]
    div = np.exp(np.arange(0, 2 * HALF, 2, dtype=np.float32)
                 * np.float32(-math.log(ROPE_BASE) / (2 * HALF)))
    ang = pos * div[None, :]
    cosv = np.cos(ang).astype(np.float32)   # [T, HALF]
    sinv = np.sin(ang).astype(np.float32)
    cosT = np.ascontiguousarray(cosv.T)     # [HALF, T]
    sinT = np.ascontiguousarray(sinv.T)
    cs2 = np.ascontiguousarray(np.concatenate([cosT, cosT], axis=0))  # [P, T]
    sn1 = sinT
    # triangle mask M[s, w] = 1 iff s <= w
    ww = np.arange(P)[None, :]
    ss = np.arange(P)[:, None]
    maskM = (ss <= ww).astype(np.float32)
    return cs2, sn1, maskM


def _make_in_maps(x, Wqkv, Wout):
    x = np.asarray(x, dtype=np.float32)
    Wqkv = np.asarray(Wqkv, dtype=np.float32)
    Wout = np.asarray(Wout, dtype=np.float32)
    assert x.shape == (B, T, C) and Wqkv.shape == (C, 3 * C) \
        and Wout.shape == (C, C)

    cs2, sn1, maskM = _host_tables()
    # xTt[b, ti, p, ko, u] = x[b, ti*TQ+u, ko*128+p]
    xTt = np.ascontiguousarray(
        x.reshape(B, NT, TQ, KO, P).transpose(0, 1, 4, 3, 2))

    in_maps = []
    for core in range(NCORES):
        h0 = core * HPC
        cols = slice(h0 * D, (h0 + HPC) * D)
        ws = []
        for part in range(3):
            w = Wqkv[:, part * C:(part + 1) * C][:, cols]  # [C, HPC*D]
            ws.append(np.ascontiguousarray(
                w.reshape(KO, P, HPC * D).transpose(1, 0, 2)))
        in_maps.append({
            "xTt": xTt,
            "wq": ws[0], "wk": ws[1], "wv": ws[2],
            "wout": Wout,
            "cs2": cs2, "sn1": sn1, "maskM": maskM,
        })
    return in_maps


def _run(x, Wqkv, Wout, trace=False):
    nc = _get_nc()
    in_maps = _make_in_maps(x, Wqkv, Wout)
    res = run_bass_kernel_spmd(nc, in_maps, core_ids=list(range(NCORES)),
                               trace=trace)
    out = np.empty((B, T, C), dtype=np.float32)
    for core in range(NCORES):
        out[:, core * HPC * D:(core + 1) * HPC * D, :] = \
            res.results[core]["y"]
    return out, res


def kernel(x, Wqkv, Wout):
    out, _ = _run(x, Wqkv, Wout)
    return out



# revision 4
# speedup vs baseline: 1.2543x; 1.2543x over previous
"""Trainium2 Bass kernel for nn_MultiHeadAttention_63015760167496.

Computation (see reference): qkv = x @ Wqkv; RoPE on q,k; causal softmax
attention per head; out = einsum('bhts,bshd->bhtd', probs, v);
out.reshape(B,T,C) @ Wout  -- NOTE the reshape is a *head-major* flatten of
[B,H,T,D] into [B,T,C], so final-output row r = h*128 + t//16 depends only on
head h.  Sharding: head-parallel over 8 cores (2 heads/core); every core
computes its two heads end-to-end and produces final-output rows
[256*i, 256*i+256).  Host concatenates -- no collectives.

All matmul operands are bf16 (fp32 PSUM accumulation): bf16 weight loads use
FWL (2 elem/cycle) so LDWEIGHTS fully hides under the matmul streaming, bf16
moving always runs 1 cycle/row, and DMA traffic halves vs fp32.  Attention is
computed in S^T layout ([s,t]); the softmax denominator is accumulated on the
vector engine (partial sums of exp tiles) and reduced with ONE ones-vector
matmul per output tile instead of one per s-chunk.  Normalization via a K=1
broadcast matmul.  No running max is needed (scores are O(5), fp32 psum).
"""

import math
import sys

for _p in ("/opt/trn_rl_repo", "/root/.axon_site/_ro/trn_rl_repo"):
    if _p not in sys.path:
        sys.path.insert(0, _p)

import numpy as np
import ml_dtypes

import concourse.bass as bass
import concourse.mybir as mybir
import concourse.tile as tile
from concourse import bacc
from concourse.bass_utils import run_bass_kernel_spmd

B, T, C = 2, 2048, 2048
H = 16            # heads total
D = C // H        # 128 head dim
HALF = D // 2     # 64
P = 128
KO = C // P       # 16 contraction chunks
NCORES = 8
HPC = H // NCORES  # 2 heads per core
TQ = 256          # t-tile for qkv projection
NT = T // TQ
TA = 512          # t-tile for attention
NTA = T // TA
NSC = T // P      # 16 s-chunks
TC = 512          # col-tile for output projection
ROPE_BASE = 10000.0
SCALE = 1.0 / math.sqrt(D)

f32 = mybir.dt.float32
f32r = mybir.dt.float32r
bf16 = mybir.dt.bfloat16
nbf16 = ml_dtypes.bfloat16


def _build():
    nc = bacc.Bacc("TRN2", target_bir_lowering=False, debug=False,
                   num_devices=NCORES)

    # host-pre-tiled x^T: xTt[b, ti, p, ko, u] = x[b, ti*TQ+u, ko*128+p]
    xTt = nc.dram_tensor("xTt", [B, NT, P, KO, TQ], bf16, kind="ExternalInput")
    # host-pre-chunked weights: w[p, ko, m] = W[ko*128+p, m]
    wq = nc.dram_tensor("wq", [P, KO, HPC * D], bf16, kind="ExternalInput")
    wk = nc.dram_tensor("wk", [P, KO, HPC * D], bf16, kind="ExternalInput")
    wv = nc.dram_tensor("wv", [P, KO, HPC * D], bf16, kind="ExternalInput")
    wout = nc.dram_tensor("wout", [P, KO, C], bf16, kind="ExternalInput")
    cs2 = nc.dram_tensor("cs2", [P, T], f32, kind="ExternalInput")  # [cos;cos]
    sn1 = nc.dram_tensor("sn1", [HALF, T], f32, kind="ExternalInput")  # sin
    maskM = nc.dram_tensor("maskM", [P, P], bf16, kind="ExternalInput")
    y = nc.dram_tensor("y", [B, HPC * D, C], f32, kind="ExternalOutput")

    with tile.TileContext(nc) as tc:
        with tc.tile_pool(name="const", bufs=1) as cp_, \
             tc.tile_pool(name="qkv", bufs=1) as qp, \
             tc.tile_pool(name="ot", bufs=1) as op_, \
             tc.tile_pool(name="small", bufs=2) as sp:

            wq_sb = cp_.tile([P, KO, HPC * D], bf16, tag="wq")
            wk_sb = cp_.tile([P, KO, HPC * D], bf16, tag="wk")
            wv_sb = cp_.tile([P, KO, HPC * D], bf16, tag="wv")
            wout_sb = cp_.tile([P, KO, C], bf16, tag="wout")
            nc.sync.dma_start(wq_sb[:], wq.ap())
            cs_sb = cp_.tile([P, T], f32, tag="cs")
            sn_sb = cp_.tile([HALF, T], f32, tag="sn")
            mask_sb = cp_.tile([P, P], bf16, tag="mask")
            ones_f1 = cp_.tile([1, P], f32, tag="ones_f1")
            nc.vector.memset(ones_f1[:], 1.0)
            ones_rowr = cp_.tile([1, P], f32r, tag="ones_rowr")
            nc.vector.tensor_copy(ones_rowr[:], ones_f1[:])
            ones_f32 = cp_.tile([P, 1], f32, tag="ones_f32")
            nc.vector.memset(ones_f32[:], 1.0)
            ones_col = cp_.tile([P, 1], f32r, tag="ones_col")
            nc.vector.tensor_copy(ones_col[:], ones_f32[:])

            # persistent attention outputs O^T per (b, local head): [d, t]
            oT = [[op_.tile([P, T], bf16, tag=f"oT{b}{hh}", name=f"oT{b}{hh}")
                   for hh in range(HPC)] for b in range(B)]

            for b in range(B):
                qT = [qp.tile([P, T], bf16, tag=f"qT{hh}", name=f"qT{b}{hh}")
                      for hh in range(HPC)]
                kT = [qp.tile([P, T], bf16, tag=f"kT{hh}", name=f"kT{b}{hh}")
                      for hh in range(HPC)]
                vt = [qp.tile([P, NSC, D], bf16, tag=f"v{hh}", name=f"v{b}{hh}")
                      for hh in range(HPC)]

                # ---------------- QKV projection + RoPE ----------------
                with tc.tile_pool(name=f"xt{b}", bufs=2) as xp, \
                     tc.tile_pool(name=f"psA{b}", bufs=4, space="PSUM") as psa, \
                     tc.tile_pool(name=f"psV{b}", bufs=3, space="PSUM") as psv_p, \
                     tc.tile_pool(name=f"rope{b}", bufs=3) as rp:
                    for ti in range(NT):
                        sl = slice(ti * TQ, (ti + 1) * TQ)
                        xt = xp.tile([P, KO, TQ], bf16, tag="xt")
                        nc.sync.dma_start(xt[:], xTt.ap()[b, ti])
                        cs = cs_sb[:, sl]
                        sn = sn_sb[:, sl]  # [64, TQ] base partition 0

                        def qkmm(w_sb, hh):
                            hsl = slice(hh * D, (hh + 1) * D)
                            ps = psa.tile([P, TQ], f32, tag="acc",
                                          name=f"acc{b}_{ti}_{hh}")
                            for ko in range(KO):
                                nc.tensor.matmul(ps[:], w_sb[:, ko, hsl],
                                                 xt[:, ko, :],
                                                 start=(ko == 0),
                                                 stop=(ko == KO - 1))
                            return ps

                        def rope(ps, dst):
                            # tcos = ps * [cos;cos] (one full mult); tsw
                            # pre-swaps halves: tsw[0:64]=q2*sin,
                            # tsw[64:128]=q1*sin so the add/sub reads align
                            # on base partitions.
                            tcos = rp.tile([P, TQ], f32, tag="tcos")
                            tsw = rp.tile([P, TQ], f32, tag="tsw")
                            nc.vector.tensor_mul(tcos[:], ps[:], cs)
                            nc.vector.tensor_mul(tsw[0:HALF, :],
                                                 ps[HALF:P, :], sn)
                            nc.vector.tensor_mul(tsw[HALF:P, :],
                                                 ps[0:HALF, :], sn)
                            nc.gpsimd.tensor_sub(dst[0:HALF, sl],
                                                 tcos[0:HALF, :],
                                                 tsw[0:HALF, :])
                            nc.gpsimd.tensor_add(dst[HALF:P, sl],
                                                 tcos[HALF:P, :],
                                                 tsw[HALF:P, :])

                        if b == 0 and ti == 0:
                            # q accums first (need only wq + xt0); stagger the
                            # remaining const DMAs behind them so the first
                            # matmuls aren't starved for DMA bandwidth.
                            psq = [qkmm(wq_sb, hh) for hh in range(HPC)]
                            nc.sync.dma_start(cs_sb[:], cs2.ap())
                            nc.sync.dma_start(sn_sb[:], sn1.ap())
                            nc.sync.dma_start(wk_sb[:], wk.ap())
                            nc.sync.dma_start(wv_sb[:], wv.ap())
                            nc.sync.dma_start(mask_sb[:], maskM.ap())
                            for hh in range(HPC):
                                rope(psq[hh], qT[hh])
                                psk = qkmm(wk_sb, hh)
                                rope(psk, kT[hh])
                        else:
                            if b == 0 and ti == 2:
                                # big out-projection weight load: needed only
                                # much later, stream it behind the hot loads
                                nc.sync.dma_start(wout_sb[:], wout.ap())
                            for hh in range(HPC):
                                for w_sb, dst in ((wq_sb, qT[hh]),
                                                  (wk_sb, kT[hh])):
                                    rope(qkmm(w_sb, hh), dst)
                        for sub in range(TQ // P):
                            psv = psv_p.tile([P, HPC * D], f32, tag="v")
                            for ko in range(KO):
                                nc.tensor.matmul(
                                    psv[:], xt[:, ko, sub * P:(sub + 1) * P],
                                    wv_sb[:, ko, :],
                                    start=(ko == 0), stop=(ko == KO - 1))
                            tci = ti * (TQ // P) + sub
                            for hh in range(HPC):
                                nc.vector.tensor_copy(
                                    vt[hh][:, tci, :],
                                    psv[:, hh * D:(hh + 1) * D])

                # ---------------- attention (S^T layout) ----------------
                with tc.tile_pool(name=f"psBsc{b}", bufs=4, space="PSUM") as pssc, \
                     tc.tile_pool(name=f"psBo{b}", bufs=2, space="PSUM") as pso, \
                     tc.tile_pool(name=f"psBsum{b}", bufs=1, space="PSUM") as pssum, \
                     tc.tile_pool(name=f"pt{b}", bufs=3) as ptp, \
                     tc.tile_pool(name=f"acc{b}", bufs=2) as accp:
                    for hh in range(HPC):
                        for ta in range(NTA):
                            ps_o = pso.tile([P, TA], f32, tag="o")
                            acc = accp.tile([P, TA], f32r, tag="acc")
                            smax = (ta + 1) * (TA // P) - 1
                            for s in range(smax + 1):
                                diag = s >= ta * (TA // P)
                                t_lo = (s - ta * (TA // P)) * P if diag else 0
                                w = slice(t_lo, TA)
                                qsl = slice(ta * TA + t_lo, (ta + 1) * TA)
                                ps_sc = pssc.tile([P, TA], f32, tag="sc")
                                nc.tensor.matmul(
                                    ps_sc[:, w], kT[hh][:, s * P:(s + 1) * P],
                                    qT[hh][:, qsl], start=True, stop=True)
                                pt = ptp.tile([P, TA], bf16, tag="pt")
                                nc.scalar.activation(
                                    pt[:, w], ps_sc[:, w],
                                    mybir.ActivationFunctionType.Exp,
                                    scale=SCALE)
                                if diag:  # mask the 128x128 triangle
                                    nc.vector.tensor_mul(
                                        pt[:, t_lo:t_lo + P],
                                        pt[:, t_lo:t_lo + P], mask_sb[:])
                                first, last = (s == 0), (s == smax)
                                nc.tensor.matmul(ps_o[:, w], vt[hh][:, s, :],
                                                 pt[:, w],
                                                 start=first, stop=last)
                                # denominator partial sums on the DVE instead
                                # of a per-s ones-matmul on the PE
                                if first:
                                    nc.vector.tensor_copy(acc[:], pt[:])
                                else:
                                    nc.vector.tensor_add(acc[:, w],
                                                         acc[:, w], pt[:, w])
                            ps_sum = pssum.tile([1, TA], f32, tag="sum")
                            nc.tensor.matmul(ps_sum[:], ones_col[:], acc[:],
                                             start=True, stop=True)
                            recf = sp.tile([1, TA], f32, tag="recf")
                            nc.vector.reciprocal_approx_fast(recf[:], ps_sum[:])
                            rec = sp.tile([1, TA], f32r, tag="rec")
                            nc.vector.tensor_copy(rec[:], recf[:])
                            ps_bc = pssc.tile([P, TA], f32, tag="sc")
                            nc.tensor.matmul(ps_bc[:], ones_rowr[:], rec[:],
                                             start=True, stop=True)
                            bc_sb = sp.tile([P, TA], f32, tag="bc_sb")
                            nc.vector.tensor_copy(bc_sb[:], ps_bc[:])
                            # write oT pre-shuffled for the out-projection:
                            # oT[p, j*128+u] = O^T[p, t=u*16+j]
                            oview = oT[b][hh].rearrange(
                                "p (j u) -> p u j", j=KO)[
                                :, (TA // 16) * ta:(TA // 16) * (ta + 1), :]
                            nc.vector.tensor_mul(
                                oview,
                                ps_o[:].rearrange("p (u j) -> p u j", j=KO),
                                bc_sb[:].rearrange("p (u j) -> p u j", j=KO))

            # ---------------- output projection ----------------
            with tc.tile_pool(name="psC", bufs=4, space="PSUM") as psc, \
                 tc.tile_pool(name="yp", bufs=3) as yp:
                for b in range(B):
                    for hh in range(HPC):
                        for cpi in range(C // TC):
                            csl = slice(cpi * TC, (cpi + 1) * TC)
                            psy = psc.tile([P, TC], f32, tag="y")
                            for j in range(KO):
                                nc.tensor.matmul(psy[:],
                                                 oT[b][hh][:, j * P:(j + 1) * P],
                                                 wout_sb[:, j, csl],
                                                 start=(j == 0),
                                                 stop=(j == KO - 1))
                            ysb = yp.tile([P, TC], f32, tag="ysb")
                            nc.scalar.copy(ysb[:], psy[:])
                            nc.sync.dma_start(
                                y.ap()[b, hh * D:(hh + 1) * D, csl], ysb[:])

    nc.compile()
    return nc


_NC = None


def _get_nc():
    global _NC
    if _NC is None:
        _NC = _build()
    return _NC


def _host_tables():
    pos = np.arange(T, dtype=np.float32)[:, None]
    div = np.exp(np.arange(0, 2 * HALF, 2, dtype=np.float32)
                 * np.float32(-math.log(ROPE_BASE) / (2 * HALF)))
    ang = pos * div[None, :]
    cosv = np.cos(ang).astype(np.float32)   # [T, HALF]
    sinv = np.sin(ang).astype(np.float32)
    cosT = np.ascontiguousarray(cosv.T)     # [HALF, T]
    sinT = np.ascontiguousarray(sinv.T)
    cs2 = np.ascontiguousarray(np.concatenate([cosT, cosT], axis=0))  # [P, T]
    sn1 = sinT
    # triangle mask M[s, w] = 1 iff s <= w
    ww = np.arange(P)[None, :]
    ss = np.arange(P)[:, None]
    maskM = (ss <= ww).astype(nbf16)
    return cs2, sn1, maskM


def _make_in_maps(x, Wqkv, Wout):
    x = np.asarray(x, dtype=np.float32)
    Wqkv = np.asarray(Wqkv, dtype=np.float32)
    Wout = np.asarray(Wout, dtype=np.float32)
    assert x.shape == (B, T, C) and Wqkv.shape == (C, 3 * C) \
        and Wout.shape == (C, C)

    cs2, sn1, maskM = _host_tables()
    # xTt[b, ti, p, ko, u] = x[b, ti*TQ+u, ko*128+p]
    xTt = np.ascontiguousarray(
        x.reshape(B, NT, TQ, KO, P).transpose(0, 1, 4, 3, 2).astype(nbf16))
    # wout[p, j, n] = Wout[j*128+p, n]
    woutT = np.ascontiguousarray(
        Wout.reshape(KO, P, C).transpose(1, 0, 2).astype(nbf16))

    in_maps = []
    for core in range(NCORES):
        h0 = core * HPC
        cols = slice(h0 * D, (h0 + HPC) * D)
        ws = []
        for part in range(3):
            w = Wqkv[:, part * C:(part + 1) * C][:, cols]  # [C, HPC*D]
            ws.append(np.ascontiguousarray(
                w.reshape(KO, P, HPC * D).transpose(1, 0, 2).astype(nbf16)))
        in_maps.append({
            "xTt": xTt,
            "wq": ws[0], "wk": ws[1], "wv": ws[2],
            "wout": woutT,
            "cs2": cs2, "sn1": sn1, "maskM": maskM,
        })
    return in_maps


def _run(x, Wqkv, Wout, trace=False):
    nc = _get_nc()
    in_maps = _make_in_maps(x, Wqkv, Wout)
    res = run_bass_kernel_spmd(nc, in_maps, core_ids=list(range(NCORES)),
                               trace=trace)
    out = np.empty((B, T, C), dtype=np.float32)
    for core in range(NCORES):
        out[:, core * HPC * D:(core + 1) * HPC * D, :] = \
            res.results[core]["y"]
    return out, res


def kernel(x, Wqkv, Wout):
    out, _ = _run(x, Wqkv, Wout)
    return out


# revision 6
# speedup vs baseline: 1.4572x; 1.1618x over previous
"""Trainium2 Bass kernel for nn_MultiHeadAttention_63015760167496.

Computation (see reference): qkv = x @ Wqkv; RoPE on q,k; causal softmax
attention per head; out = einsum('bhts,bshd->bhtd', probs, v);
out.reshape(B,T,C) @ Wout  -- NOTE the reshape is a *head-major* flatten of
[B,H,T,D] into [B,T,C], so final-output row r = h*128 + t//16 depends only on
head h.  Sharding: head-parallel over 8 cores (2 heads/core); every core
computes its two heads end-to-end and produces final-output rows
[256*i, 256*i+256).  Host concatenates -- no collectives.

All matmul operands are bf16 (fp32 PSUM accumulation): bf16 weight loads use
FWL so LDWEIGHTS hides under matmul streaming, and DMA traffic halves vs
fp32.  Attention runs in S^T layout ([s,t]) and is interleaved into the QKV
projection stream: attention t-tile `ta` only needs tokens < 512*(ta+1), so
it is emitted right after projection chunk `ta`, letting the scalar-engine
exp (the attention pacer) overlap projection matmuls on the PE.  Softmax
denominator via ones-vector matmul accumulated in PSUM over the s-loop;
normalization via a K=1 broadcast matmul.  No running max is needed (scores
are O(5), fp32 psum).
"""

import math
import sys

for _p in ("/opt/trn_rl_repo", "/root/.axon_site/_ro/trn_rl_repo"):
    if _p not in sys.path:
        sys.path.insert(0, _p)

import numpy as np
import ml_dtypes

import concourse.bass as bass
import concourse.mybir as mybir
import concourse.tile as tile
from concourse import bacc
from concourse.bass_utils import run_bass_kernel_spmd

B, T, C = 2, 2048, 2048
H = 16            # heads total
D = C // H        # 128 head dim
HALF = D // 2     # 64
P = 128
KO = C // P       # 16 contraction chunks
NCORES = 8
HPC = H // NCORES  # 2 heads per core
TQ = 512          # t-tile for qkv projection == attention t-tile
NT = T // TQ      # 4
TA = 512
NTA = T // TA     # 4
NSC = T // P      # 16 s-chunks
TC = 512          # col-tile for output projection
ROPE_BASE = 10000.0
SCALE = 1.0 / math.sqrt(D)

f32 = mybir.dt.float32
f32r = mybir.dt.float32r
bf16 = mybir.dt.bfloat16
nbf16 = ml_dtypes.bfloat16


def _build():
    nc = bacc.Bacc("TRN2", target_bir_lowering=False, debug=False,
                   num_devices=NCORES)

    # host-pre-tiled x^T: xTt[b, ti, p, ko, u] = x[b, ti*TQ+u, ko*128+p]
    xTt = nc.dram_tensor("xTt", [B, NT, P, KO, TQ], bf16, kind="ExternalInput")
    # host-pre-chunked weights: w[p, ko, m] = W[ko*128+p, m]
    wq = nc.dram_tensor("wq", [P, KO, HPC * D], bf16, kind="ExternalInput")
    wk = nc.dram_tensor("wk", [P, KO, HPC * D], bf16, kind="ExternalInput")
    wv = nc.dram_tensor("wv", [P, KO, HPC * D], bf16, kind="ExternalInput")
    wout = nc.dram_tensor("wout", [P, KO, C], bf16, kind="ExternalInput")
    cs2 = nc.dram_tensor("cs2", [P, T], bf16, kind="ExternalInput")  # [cos;cos]
    sn1 = nc.dram_tensor("sn1", [HALF, T], bf16, kind="ExternalInput")  # sin
    maskM = nc.dram_tensor("maskM", [P, P], bf16, kind="ExternalInput")
    y = nc.dram_tensor("y", [B, HPC * D, C], f32, kind="ExternalOutput")

    with tile.TileContext(nc) as tc:
        with tc.tile_pool(name="const", bufs=1) as cp_, \
             tc.tile_pool(name="qkv", bufs=1) as qp, \
             tc.tile_pool(name="ot", bufs=1) as op_, \
             tc.tile_pool(name="small", bufs=2) as sp:

            wq_sb = cp_.tile([P, KO, HPC * D], bf16, tag="wq")
            wk_sb = cp_.tile([P, KO, HPC * D], bf16, tag="wk")
            wv_sb = cp_.tile([P, KO, HPC * D], bf16, tag="wv")
            wout_sb = cp_.tile([P, KO, C], bf16, tag="wout")
            cs_sb = cp_.tile([P, T], bf16, tag="cs")
            sn_sb = cp_.tile([HALF, T], bf16, tag="sn")
            mask_sb = cp_.tile([P, P], bf16, tag="mask")
            ones_f1 = cp_.tile([1, P], f32, tag="ones_f1")
            nc.vector.memset(ones_f1[:], 1.0)
            ones_rowr = cp_.tile([1, P], f32r, tag="ones_rowr")
            nc.vector.tensor_copy(ones_rowr[:], ones_f1[:])
            ones_f32 = cp_.tile([P, 1], f32, tag="ones_f32")
            nc.vector.memset(ones_f32[:], 1.0)
            ones_col = cp_.tile([P, 1], bf16, tag="ones_col")
            nc.vector.tensor_copy(ones_col[:], ones_f32[:])

            # persistent attention outputs O^T per (b, local head): [d, t]
            oT = [[op_.tile([P, T], bf16, tag=f"oT{b}{hh}", name=f"oT{b}{hh}")
                   for hh in range(HPC)] for b in range(B)]

            for b in range(B):
                qT = [qp.tile([P, T], bf16, tag=f"qT{hh}", name=f"qT{b}{hh}")
                      for hh in range(HPC)]
                kT = [qp.tile([P, T], bf16, tag=f"kT{hh}", name=f"kT{b}{hh}")
                      for hh in range(HPC)]
                # both heads interleaved: vt[:, s, hh*D:(hh+1)*D]
                vt = qp.tile([P, NSC, HPC * D], bf16, tag="vt", name=f"v{b}")

                with tc.tile_pool(name=f"xt{b}", bufs=2) as xp, \
                     tc.tile_pool(name=f"psA{b}", bufs=2, space="PSUM") as psa, \
                     tc.tile_pool(name=f"psV{b}", bufs=1, space="PSUM") as psvp, \
                     tc.tile_pool(name=f"psSC{b}", bufs=2, space="PSUM") as pssc, \
                     tc.tile_pool(name=f"psO{b}", bufs=2, space="PSUM") as pso, \
                     tc.tile_pool(name=f"psSum{b}", bufs=1, space="PSUM") as pssum, \
                     tc.tile_pool(name=f"rope{b}", bufs=2) as rp, \
                     tc.tile_pool(name=f"pt{b}", bufs=3) as ptp:

                    def rope(ps, dst, sl, cs, sn):
                        # tcos = ps * [cos;cos] (one full mult); tsw pre-swaps
                        # halves: tsw[0:64]=q2*sin, tsw[64:128]=q1*sin so the
                        # add/sub reads align on base partitions.
                        tcos = rp.tile([P, TQ], f32, tag="tcos")
                        tsw = rp.tile([P, TQ], f32, tag="tsw")
                        nc.vector.tensor_mul(tcos[:], ps[:], cs)
                        nc.vector.tensor_mul(tsw[0:HALF, :], ps[HALF:P, :], sn)
                        nc.vector.tensor_mul(tsw[HALF:P, :], ps[0:HALF, :], sn)
                        nc.gpsimd.tensor_sub(dst[0:HALF, sl],
                                             tcos[0:HALF, :], tsw[0:HALF, :])
                        nc.gpsimd.tensor_add(dst[HALF:P, sl],
                                             tcos[HALF:P, :], tsw[HALF:P, :])

                    def qkv_chunk(ti):
                        sl = slice(ti * TQ, (ti + 1) * TQ)
                        xt = xp.tile([P, KO, TQ], bf16, tag="xt")
                        if b == 0 and ti == 0:
                            # split first load so matmuls start sooner, and
                            # stagger the other const loads behind it
                            nc.sync.dma_start(wq_sb[:], wq.ap())
                            nc.sync.dma_start(xt[:, 0:KO // 2, :],
                                              xTt.ap()[b, ti, :, 0:KO // 2])
                            nc.sync.dma_start(xt[:, KO // 2:KO, :],
                                              xTt.ap()[b, ti, :, KO // 2:KO])
                            nc.sync.dma_start(wk_sb[:], wk.ap())
                            nc.sync.dma_start(wv_sb[:], wv.ap())
                            nc.sync.dma_start(cs_sb[:], cs2.ap())
                            nc.sync.dma_start(sn_sb[:], sn1.ap())
                            nc.sync.dma_start(mask_sb[:], maskM.ap())
                        else:
                            nc.sync.dma_start(xt[:], xTt.ap()[b, ti])
                        cs = cs_sb[:, sl]
                        sn = sn_sb[:, sl]
                        for hh in range(HPC):
                            hsl = slice(hh * D, (hh + 1) * D)
                            for w_sb, dst in ((wq_sb, qT[hh]), (wk_sb, kT[hh])):
                                ps = psa.tile([P, TQ], f32, tag="qk")
                                for ko in range(KO):
                                    nc.tensor.matmul(ps[:], w_sb[:, ko, hsl],
                                                     xt[:, ko, :],
                                                     start=(ko == 0),
                                                     stop=(ko == KO - 1))
                                rope(ps, dst, sl, cs, sn)
                        for sub in range(TQ // P):
                            psv = psvp.tile([P, HPC * D], f32, tag="v")
                            for ko in range(KO):
                                nc.tensor.matmul(
                                    psv[:], xt[:, ko, sub * P:(sub + 1) * P],
                                    wv_sb[:, ko, :],
                                    start=(ko == 0), stop=(ko == KO - 1))
                            tci = ti * (TQ // P) + sub
                            nc.vector.tensor_copy(vt[:, tci, :], psv[:])
                        if b == 0 and ti == 1:
                            # big out-projection weight load: needed much
                            # later, stream it behind the hot loads
                            nc.sync.dma_start(wout_sb[:], wout.ap())

                    def attn_tile(hh, ta):
                        ps_o = pso.tile([P, TA], f32, tag="o")
                        ps_sum = pssum.tile([1, TA], f32, tag="sum")
                        smax = (ta + 1) * (TA // P) - 1
                        for s in range(smax + 1):
                            diag = s >= ta * (TA // P)
                            t_lo = (s - ta * (TA // P)) * P if diag else 0
                            w = slice(t_lo, TA)
                            qsl = slice(ta * TA + t_lo, (ta + 1) * TA)
                            ps_sc = pssc.tile([P, TA], f32, tag="sc")
                            nc.tensor.matmul(
                                ps_sc[:, w], kT[hh][:, s * P:(s + 1) * P],
                                qT[hh][:, qsl], start=True, stop=True)
                            pt = ptp.tile([P, TA], bf16, tag="pt")
                            nc.scalar.activation(
                                pt[:, w], ps_sc[:, w],
                                mybir.ActivationFunctionType.Exp,
                                scale=SCALE)
                            if diag:  # mask the 128x128 triangle
                                nc.vector.tensor_mul(
                                    pt[:, t_lo:t_lo + P],
                                    pt[:, t_lo:t_lo + P], mask_sb[:])
                            first, last = (s == 0), (s == smax)
                            nc.tensor.matmul(ps_o[:, w],
                                             vt[:, s, hh * D:(hh + 1) * D],
                                             pt[:, w], start=first, stop=last)
                            nc.tensor.matmul(ps_sum[:, w], ones_col[:],
                                             pt[:, w], start=first, stop=last)
                        recf = sp.tile([1, TA], f32, tag="recf")
                        nc.vector.reciprocal_approx_fast(recf[:], ps_sum[:])
                        rec = sp.tile([1, TA], f32r, tag="rec")
                        nc.vector.tensor_copy(rec[:], recf[:])
                        ps_bc = pssc.tile([P, TA], f32, tag="sc")
                        nc.tensor.matmul(ps_bc[:], ones_rowr[:], rec[:],
                                         start=True, stop=True)
                        bc_sb = sp.tile([P, TA], f32, tag="bc_sb")
                        nc.vector.tensor_copy(bc_sb[:], ps_bc[:])
                        # write oT pre-shuffled for the out-projection:
                        # oT[p, j*128+u] = O^T[p, t=u*16+j]
                        oview = oT[b][hh].rearrange(
                            "p (j u) -> p u j", j=KO)[
                            :, (TA // 16) * ta:(TA // 16) * (ta + 1), :]
                        nc.vector.tensor_mul(
                            oview,
                            ps_o[:].rearrange("p (u j) -> p u j", j=KO),
                            bc_sb[:].rearrange("p (u j) -> p u j", j=KO))

                    # attention tile `ta` only needs projected tokens
                    # t < (ta+1)*512, i.e. chunks 0..ta: interleave so the
                    # scalar-engine exp overlaps projection matmuls
                    for ti in range(NT):
                        qkv_chunk(ti)
                        for hh in range(HPC):
                            attn_tile(hh, ti)

            # ---------------- output projection ----------------
            with tc.tile_pool(name="psC", bufs=4, space="PSUM") as psc, \
                 tc.tile_pool(name="yp", bufs=3) as yp:
                for b in range(B):
                    for hh in range(HPC):
                        for cpi in range(C // TC):
                            csl = slice(cpi * TC, (cpi + 1) * TC)
                            psy = psc.tile([P, TC], f32, tag="y")
                            for j in range(KO):
                                nc.tensor.matmul(psy[:],
                                                 oT[b][hh][:, j * P:(j + 1) * P],
                                                 wout_sb[:, j, csl],
                                                 start=(j == 0),
                                                 stop=(j == KO - 1))
                            ysb = yp.tile([P, TC], f32, tag="ysb")
                            nc.scalar.copy(ysb[:], psy[:])
                            nc.sync.dma_start(
                                y.ap()[b, hh * D:(hh + 1) * D, csl], ysb[:])

    nc.compile()
    return nc


_NC = None


def _get_nc():
    global _NC
    if _NC is None:
        _NC = _build()
    return _NC


def _host_tables():
    pos = np.arange(T, dtype=np.float32)[:, None]
    div = np.exp(np.arange(0, 2 * HALF, 2, dtype=np.float32)
                 * np.float32(-math.log(ROPE_BASE) / (2 * HALF)))
    ang = pos * div[None, :]
    cosv = np.cos(ang).astype(np.float32)   # [T, HALF]
    sinv = np.sin(ang).astype(np.float32)
    cosT = np.ascontiguousarray(cosv.T)     # [HALF, T]
    sinT = np.ascontiguousarray(sinv.T)
    cs2 = np.ascontiguousarray(
        np.concatenate([cosT, cosT], axis=0)).astype(nbf16)  # [P, T]
    sn1 = sinT.astype(nbf16)
    # triangle mask M[s, w] = 1 iff s <= w
    ww = np.arange(P)[None, :]
    ss = np.arange(P)[:, None]
    maskM = (ss <= ww).astype(nbf16)
    return cs2, sn1, maskM


def _make_in_maps(x, Wqkv, Wout):
    x = np.asarray(x, dtype=np.float32)
    Wqkv = np.asarray(Wqkv, dtype=np.float32)
    Wout = np.asarray(Wout, dtype=np.float32)
    assert x.shape == (B, T, C) and Wqkv.shape == (C, 3 * C) \
        and Wout.shape == (C, C)

    cs2, sn1, maskM = _host_tables()
    # xTt[b, ti, p, ko, u] = x[b, ti*TQ+u, ko*128+p]
    xTt = np.ascontiguousarray(
        x.reshape(B, NT, TQ, KO, P).transpose(0, 1, 4, 3, 2).astype(nbf16))
    # wout[p, j, n] = Wout[j*128+p, n]
    woutT = np.ascontiguousarray(
        Wout.reshape(KO, P, C).transpose(1, 0, 2).astype(nbf16))

    in_maps = []
    for core in range(NCORES):
        h0 = core * HPC
        cols = slice(h0 * D, (h0 + HPC) * D)
        ws = []
        for part in range(3):
            w = Wqkv[:, part * C:(part + 1) * C][:, cols]  # [C, HPC*D]
            ws.append(np.ascontiguousarray(
                w.reshape(KO, P, HPC * D).transpose(1, 0, 2).astype(nbf16)))
        in_maps.append({
            "xTt": xTt,
            "wq": ws[0], "wk": ws[1], "wv": ws[2],
            "wout": woutT,
            "cs2": cs2, "sn1": sn1, "maskM": maskM,
        })
    return in_maps


def _run(x, Wqkv, Wout, trace=False):
    nc = _get_nc()
    in_maps = _make_in_maps(x, Wqkv, Wout)
    res = run_bass_kernel_spmd(nc, in_maps, core_ids=list(range(NCORES)),
                               trace=trace)
    out = np.empty((B, T, C), dtype=np.float32)
    for core in range(NCORES):
        out[:, core * HPC * D:(core + 1) * HPC * D, :] = \
            res.results[core]["y"]
    return out, res


def kernel(x, Wqkv, Wout):
    out, _ = _run(x, Wqkv, Wout)
    return out
